# revision 14
# baseline (speedup 1.0000x reference)
"""Cross-attention block kernel for 8 TRN2 NeuronCores.

Math (per batch element b, one per core):
    Q = q @ Wq^T            [Lq, 128]
    K = k @ Wk^T            [Lkv, 128]
    V = v @ Wv^T            [Lkv, 128]
    S = Q @ K^T * d^-0.5    [Lq, Lkv]
    O = softmax(S) @ V      [Lq, 128]

Device strategy (per core):
  - HWDGE DMA loads q/k/v fp32 from HBM in 512-row slabs, natural layout.
  - PE transposes (matmul against identity) produce feat-major bf16 tiles;
    the PSUM->SBUF evacuation does the fp32->bf16 cast and is split between
    the vector and scalar engines.
  - Projections: QT/KT computed as [d, seq] (weights stationary); V computed
    in natural [seq, d] layout (v^T tiles stationary, Wv^T moving).
  - S^T tiles [k,q] = KT_slice.T @ QT; softmax runs WITHOUT max subtraction
    (scores are ~N(0,1); exp is safe in fp32) so exp+scale is a single
    scalar-engine activation pass straight out of PSUM.
  - P^T tiles feed PV matmuls as the stationary operand against an
    augmented moving operand [V | 1]: the extra ones column makes the PSUM
    accumulator [q,129] hold both O_unnorm and the softmax denominator.
  - Normalization is a per-partition reciprocal + tensor_scalar multiply.
"""

import os
import numpy as np
import ml_dtypes

from contextlib import ExitStack

import concourse.bass as bass
import concourse.tile as tile
from concourse import bacc, mybir
from concourse.bass_utils import run_bass_kernel_spmd

F32 = mybir.dt.float32
BF16 = mybir.dt.bfloat16

B = 8
LQ = 2048
LKV = 2048
DQ = 512
DKV = 768
D = 128
N_CORES = 8

_cache = {}


def build_program(Lq=LQ, Lkv=LKV, Dq=DQ, Dkv=DKV):
    assert Lq % 128 == 0 and Lkv % 128 == 0 and Dq % 128 == 0 and Dkv % 128 == 0
    nc = bacc.Bacc("TRN2", target_bir_lowering=False)

    q_d = nc.declare_dram_parameter("q", [Lq, Dq], F32, isOutput=False)
    k_d = nc.declare_dram_parameter("k", [Lkv, Dkv], F32, isOutput=False)
    v_d = nc.declare_dram_parameter("v", [Lkv, Dkv], F32, isOutput=False)
    wq_d = nc.declare_dram_parameter("wqT", [Dq, D], BF16, isOutput=False)
    wk_d = nc.declare_dram_parameter("wkT", [Dkv, D], BF16, isOutput=False)
    wv_d = nc.declare_dram_parameter("wvT", [Dkv, D], BF16, isOutput=False)
    out_d = nc.declare_dram_parameter("out", [Lq, D], F32, isOutput=True)

    with tile.TileContext(nc) as tc:
        _body(tc, q_d, k_d, v_d, wq_d, wk_d, wv_d, out_d, Lq, Lkv, Dq, Dkv)
    nc.compile()
    return nc


def _body(tc, q_d, k_d, v_d, wq_d, wk_d, wv_d, out_d, Lq, Lkv, Dq, Dkv):
    nc = tc.nc
    scale = float(D) ** -0.5
    ICQ = Dq // 128   # q feature chunks
    ICK = Dkv // 128  # k/v feature chunks
    NKT = Lkv // 128  # kv seq tiles
    QCW = 512 if Lq % 512 == 0 else 128   # q chunk width for attention
    NQC = Lq // QCW
    QSUB = QCW // 128
    NQT = Lq // 128
    SLABQ = 512 if Lq % 512 == 0 else 128
    SLABK = 512 if Lkv % 512 == 0 else 128
    KTG = 4 if NKT % 4 == 0 else 1       # kv tiles per exp group

    with ExitStack() as ctx:
        # -------- SBUF pools --------
        wpool = ctx.enter_context(tc.tile_pool(name="weights", bufs=1))
        xtp = ctx.enter_context(tc.tile_pool(name="xT", bufs=1))
        projp = ctx.enter_context(tc.tile_pool(name="proj", bufs=1))
        stag = ctx.enter_context(tc.tile_pool(name="stage", bufs=3))
        ptp = ctx.enter_context(tc.tile_pool(name="probs", bufs=2))
        outp = ctx.enter_context(tc.tile_pool(name="outs", bufs=4))

        # -------- phase-1 PSUM pools (closed before attention) --------
        phase1 = ctx.enter_context(ExitStack())
        ptrp = phase1.enter_context(tc.tile_pool(name="psum_tr", bufs=2, space="PSUM"))
        psp = phase1.enter_context(tc.tile_pool(name="psum_proj", bufs=2, space="PSUM"))

        # transposed inputs, bf16: xT[p=feat128, chunk, seq]
        qT = xtp.tile([128, ICQ, Lq], BF16, name="qT")
        kT = xtp.tile([128, ICK, Lkv], BF16, name="kT")
        vT = xtp.tile([128, ICK, Lkv], BF16, name="vT")

        # projections
        QT = projp.tile([128, Lq], BF16, name="QT")     # [d, q]
        KT = projp.tile([128, Lkv], BF16, name="KT")    # [d, k]
        Vn = projp.tile([128, NKT, D + 1], BF16, name="Vn")  # natural V + ones

        # weights, already transposed on host: [Din, D] -> sbuf [128, IC, D]
        wq_sb = wpool.tile([128, ICQ, D], BF16, name="wq_sb")
        wk_sb = wpool.tile([128, ICK, D], BF16, name="wk_sb")
        wv_sb = wpool.tile([128, ICK, D], BF16, name="wv_sb")

        # identity for PE transposes
        ident = wpool.tile([128, 128], F32, name="ident")

        evac_flip = [0]

        def load_transpose(x_d, xT_sb, slab, ic_n, tag, sl):
            """Load one fp32 slab, PE-transpose its 128x128 tiles into PSUM
            (fp32), evacuate each bank to bf16 via DVE/ACT copies."""
            nt = slab // 128
            nat = stag.tile([128, nt, ic_n * 128], F32,
                            name=f"nat_{tag}_{sl}", tag="nat")
            nc.sync.dma_start(
                out=nat[:],
                in_=x_d[:][sl * slab:(sl + 1) * slab, :]
                .rearrange("(t p) i -> p t i", p=128))
            for ic in range(ic_n):
                ptr = ptrp.tile([128, slab], F32,
                                name=f"ptr_{tag}_{sl}_{ic}", tag="ptr")
                for t in range(nt):
                    nc.tensor.transpose(
                        ptr[:, t * 128:(t + 1) * 128],
                        nat[:, t, ic * 128:(ic + 1) * 128],
                        ident[:])
                dst = xT_sb[:, ic, sl * slab:(sl + 1) * slab]
                # evac with cast; DVE gets 2 of 3 (ACT is busy with exp)
                if evac_flip[0] % 3 == 2:
                    nc.scalar.copy(dst, ptr[:])
                else:
                    nc.vector.tensor_copy(dst, ptr[:])
                evac_flip[0] += 1

        def project_chunk(w_sb, x_sb, out_sb, ic_n, sc, ncols, tag):
            ps = psp.tile([128, ncols], F32, name=f"ps_{tag}_{sc}", tag="ps")
            for ic in range(ic_n):
                nc.tensor.matmul(
                    ps[:],
                    lhsT=w_sb[:, ic, :],
                    rhs=x_sb[:, ic, sc * ncols:(sc + 1) * ncols],
                    start=(ic == 0), stop=(ic == ic_n - 1))
            nc.vector.tensor_copy(out_sb[:, sc * ncols:(sc + 1) * ncols], ps[:])

        def project_v(kt):
            psv = psp.tile([128, D], F32, name=f"psv_{kt}", tag="ps")
            for ic in range(ICK):
                nc.tensor.matmul(
                    psv[:],
                    lhsT=vT[:, ic, kt * 128:(kt + 1) * 128],
                    rhs=wv_sb[:, ic, :],
                    start=(ic == 0), stop=(ic == ICK - 1))
            nc.vector.tensor_copy(Vn[:, kt, 0:D], psv[:])

        from concourse.masks import make_identity
        make_identity(nc, ident[:])
        nc.vector.memset(Vn[:], 1.0)

        # first data DMA before the small setup transfers so HBM streaming
        # starts immediately
        load_transpose(q_d, qT, SLABQ, ICQ, "q", 0)
        nc.sync.dma_start(out=wq_sb[:], in_=wq_d[:].rearrange("(c p) d -> p c d", p=128))
        nc.sync.dma_start(out=wk_sb[:], in_=wk_d[:].rearrange("(c p) d -> p c d", p=128))
        nc.sync.dma_start(out=wv_sb[:], in_=wv_d[:].rearrange("(c p) d -> p c d", p=128))

        for sl in range(1, Lq // SLABQ):
            load_transpose(q_d, qT, SLABQ, ICQ, "q", sl)
        ncq = min(512, SLABQ)
        for sc in range(Lq // ncq):
            project_chunk(wq_sb, qT, QT, ICQ, sc, ncq, "q")

        for sl in range(Lkv // SLABK):
            load_transpose(k_d, kT, SLABK, ICK, "k", sl)
            load_transpose(v_d, vT, SLABK, ICK, "v", sl)
        nck = min(512, SLABK)
        for sc in range(Lkv // nck):
            project_chunk(wk_sb, kT, KT, ICK, sc, nck, "k")
        for kt in range(NKT):
            project_v(kt)

        # -------- attention (phase-1 PSUM released first) --------
        phase1.close()
        pssp = ctx.enter_context(tc.tile_pool(name="psum_s", bufs=1, space="PSUM"))
        psop = ctx.enter_context(tc.tile_pool(name="psum_o", bufs=1, space="PSUM"))
        NG = NKT // KTG
        for qc in range(NQC):
            pso = [psop.tile([128, D + 1], F32, name=f"pso{qs}_{qc}",
                             tag=f"pso{qs}")
                   for qs in range(QSUB)]
            for g in range(NG):
                # KTG S^T tiles into one multi-bank PSUM tile, then a single
                # wide exp covering all of them
                pssg = pssp.tile([128, KTG, QCW], F32,
                                 name=f"pssg_{qc}_{g}", tag="pssg")
                for i in range(KTG):
                    kt = g * KTG + i
                    nc.tensor.matmul(
                        pssg[:, i, :],
                        lhsT=KT[:, kt * 128:(kt + 1) * 128],
                        rhs=QT[:, qc * QCW:(qc + 1) * QCW],
                        start=True, stop=True)
                pt = ptp.tile([128, KTG, QCW], BF16,
                              name=f"pt_{qc}_{g}", tag="pt")
                nc.scalar.activation(
                    pt[:], pssg[:], mybir.ActivationFunctionType.Exp,
                    scale=scale)
                for i in range(KTG):
                    kt = g * KTG + i
                    for qs in range(QSUB):
                        nc.tensor.matmul(
                            pso[qs][:],
                            lhsT=pt[:, i, qs * 128:(qs + 1) * 128],
                            rhs=Vn[:, kt, :],
                            start=(kt == 0), stop=(kt == NKT - 1))
            for qs in range(QSUB):
                acc = pso[qs][:]
                r = outp.tile([128, 1], F32, name=f"r_{qc}_{qs}", tag="r")
                nc.vector.reciprocal(r[:], acc[:, D:D + 1])
                o = outp.tile([128, D], F32, name=f"o_{qc}_{qs}", tag="o")
                nc.vector.tensor_scalar_mul(o[:], acc[:, 0:D], r[:])
                row = (qc * QSUB + qs) * 128
                nc.sync.dma_start(out=out_d[:][row:row + 128, :], in_=o[:])


def _get_program():
    key = (LQ, LKV, DQ, DKV)
    if key not in _cache:
        _cache[key] = build_program(*key)
    return _cache[key]


def kernel(q_input, k_input, v_input, Wq, Wk, Wv):
    out_dtype = q_input.dtype
    nc = _get_program()

    wqT = np.ascontiguousarray(Wq.T).astype(ml_dtypes.bfloat16)
    wkT = np.ascontiguousarray(Wk.T).astype(ml_dtypes.bfloat16)
    wvT = np.ascontiguousarray(Wv.T).astype(ml_dtypes.bfloat16)

    in_maps = []
    for c in range(N_CORES):
        in_maps.append({
            "q": np.ascontiguousarray(q_input[c]).astype(np.float32),
            "k": np.ascontiguousarray(k_input[c]).astype(np.float32),
            "v": np.ascontiguousarray(v_input[c]).astype(np.float32),
            "wqT": wqT, "wkT": wkT, "wvT": wvT,
        })

    trace = bool(int(os.environ.get("KERNEL_TRACE", "0")))
    if trace:
        try:
            import antenv.axon_hooks  # noqa: F401  (needed by the trace path)
        except ImportError:
            trace = False
    res = run_bass_kernel_spmd(nc, in_maps, list(range(N_CORES)), trace=trace)
    kernel.last_results = res

    out = np.stack([res.results[c]["out"] for c in range(N_CORES)], axis=0)
    return out.astype(out_dtype)


# revision 15
# speedup vs baseline: 1.3090x; 1.3090x over previous
"""Cross-attention block kernel for 8 TRN2 NeuronCores.

Math (per batch element b, one per core):
    Q = q @ Wq^T            [Lq, 128]
    K = k @ Wk^T            [Lkv, 128]
    V = v @ Wv^T            [Lkv, 128]
    S = Q @ K^T * d^-0.5    [Lq, Lkv]
    O = softmax(S) @ V      [Lq, 128]

Device strategy (per core):
  - HWDGE DMA loads q/k/v fp32 from HBM in 512-row slabs, natural layout.
  - PE transposes (matmul against identity) produce feat-major bf16 tiles;
    the PSUM->SBUF evacuation does the fp32->bf16 cast and is split between
    the vector and scalar engines.
  - Projections: QT/KT computed as [d, seq] (weights stationary); V computed
    in natural [seq, d] layout (v^T tiles stationary, Wv^T moving).
  - S^T tiles [k,q] = KT_slice.T @ QT; softmax runs WITHOUT max subtraction
    (scores are ~N(0,1); exp is safe in fp32) so exp+scale is a single
    scalar-engine activation pass straight out of PSUM.
  - P^T tiles feed PV matmuls as the stationary operand against an
    augmented moving operand [V | 1]: the extra ones column makes the PSUM
    accumulator [q,129] hold both O_unnorm and the softmax denominator.
  - Normalization is a per-partition reciprocal + tensor_scalar multiply.
"""

import os
import numpy as np
import ml_dtypes

from contextlib import ExitStack

import concourse.bass as bass
import concourse.tile as tile
from concourse import bacc, mybir
from concourse.bass_utils import run_bass_kernel_spmd

F32 = mybir.dt.float32
BF16 = mybir.dt.bfloat16

B = 8
LQ = 2048
LKV = 2048
DQ = 512
DKV = 768
D = 128
N_CORES = 8

_cache = {}


def build_program(Lq=LQ, Lkv=LKV, Dq=DQ, Dkv=DKV):
    assert Lq % 128 == 0 and Lkv % 128 == 0 and Dq % 128 == 0 and Dkv % 128 == 0
    nc = bacc.Bacc("TRN2", target_bir_lowering=False)

    q_d = nc.declare_dram_parameter("q", [Lq, Dq], F32, isOutput=False)
    k_d = nc.declare_dram_parameter("k", [Lkv, Dkv], F32, isOutput=False)
    v_d = nc.declare_dram_parameter("v", [Lkv, Dkv], F32, isOutput=False)
    wq_d = nc.declare_dram_parameter("wqT", [Dq, D], BF16, isOutput=False)
    wk_d = nc.declare_dram_parameter("wkT", [Dkv, D], BF16, isOutput=False)
    wv_d = nc.declare_dram_parameter("wvT", [Dkv, D], BF16, isOutput=False)
    out_d = nc.declare_dram_parameter("out", [Lq, D], F32, isOutput=True)

    with tile.TileContext(nc) as tc:
        _body(tc, q_d, k_d, v_d, wq_d, wk_d, wv_d, out_d, Lq, Lkv, Dq, Dkv)
    nc.compile()
    return nc


def _body(tc, q_d, k_d, v_d, wq_d, wk_d, wv_d, out_d, Lq, Lkv, Dq, Dkv):
    nc = tc.nc
    scale = float(D) ** -0.5
    ICQ = Dq // 128   # q feature chunks
    ICK = Dkv // 128  # k/v feature chunks
    NKT = Lkv // 128  # kv seq tiles
    QCW = 512 if Lq % 512 == 0 else 128   # q chunk width for attention
    NQC = Lq // QCW
    QSUB = QCW // 128

    with ExitStack() as ctx:
        # -------- SBUF pools --------
        wpool = ctx.enter_context(tc.tile_pool(name="weights", bufs=1))
        xtp = ctx.enter_context(tc.tile_pool(name="xT", bufs=1))
        projp = ctx.enter_context(tc.tile_pool(name="proj", bufs=1))
        stag = ctx.enter_context(tc.tile_pool(name="stage", bufs=3))
        ptp = ctx.enter_context(tc.tile_pool(name="probs", bufs=3))
        outp = ctx.enter_context(tc.tile_pool(name="outs", bufs=4))

        # weights, already transposed on host: [Din, D] -> sbuf [128, IC, D]
        wq_sb = wpool.tile([128, ICQ, D], BF16, name="wq_sb")
        wk_sb = wpool.tile([128, ICK, D], BF16, name="wk_sb")
        wv_sb = wpool.tile([128, ICK, D], BF16, name="wv_sb")
        nc.sync.dma_start(out=wq_sb[:], in_=wq_d[:].rearrange("(c p) d -> p c d", p=128))
        nc.sync.dma_start(out=wk_sb[:], in_=wk_d[:].rearrange("(c p) d -> p c d", p=128))
        nc.sync.dma_start(out=wv_sb[:], in_=wv_d[:].rearrange("(c p) d -> p c d", p=128))

        # transposed inputs, bf16: xT[p=feat128, chunk, seq]
        qT = xtp.tile([128, ICQ, Lq], BF16, name="qT")
        kT = xtp.tile([128, ICK, Lkv], BF16, name="kT")
        vT = xtp.tile([128, ICK, Lkv], BF16, name="vT")

        # projections
        QT = projp.tile([128, Lq], BF16, name="QT")     # [d, q]
        KT = projp.tile([128, Lkv], BF16, name="KT")    # [d, k]
        Vn = projp.tile([128, NKT, D + 1], BF16, name="Vn")  # natural V + ones

        # identity for PE transposes
        ident = wpool.tile([128, 128], F32, name="ident")
        from concourse.masks import make_identity
        make_identity(nc, ident[:])

        # ones column for the fused denominator trick (data cols overwritten)
        nc.vector.memset(Vn[:], 1.0)

        # -------- load + transpose + projections (phase 1 PSUM scope) ------
        evac_flip = [0]

        with tc.tile_pool(name="psum_tr", bufs=3, space="PSUM") as ptrp, \
             tc.tile_pool(name="psum_proj", bufs=2, space="PSUM") as psp:

            # Load 512-row fp32 slabs, PE-transpose 128x128 tiles into PSUM
            # (4 per bank), evacuate each bank as bf16 (cast in the copy),
            # alternating vector/scalar engines.
            def load_transpose(x_d, xT_sb, n_seq, ic_n, tag):
                slab = 512 if n_seq % 512 == 0 else 128
                nt = slab // 128
                for sl in range(n_seq // slab):
                    nat = stag.tile([128, nt, ic_n * 128], F32,
                                    name=f"nat_{tag}_{sl}", tag="nat")
                    nc.sync.dma_start(
                        out=nat[:],
                        in_=x_d[:][sl * slab:(sl + 1) * slab, :]
                        .rearrange("(t p) i -> p t i", p=128))
                    for ic in range(ic_n):
                        ptr = ptrp.tile([128, slab], F32,
                                        name=f"ptr_{tag}_{sl}_{ic}", tag="ptr")
                        for t in range(nt):
                            nc.tensor.transpose(
                                ptr[:, t * 128:(t + 1) * 128],
                                nat[:, t, ic * 128:(ic + 1) * 128],
                                ident[:])
                        dst = xT_sb[:, ic, sl * slab:(sl + 1) * slab]
                        if evac_flip[0] % 2 == 0:
                            nc.vector.tensor_copy(dst, ptr[:])
                        else:
                            nc.scalar.copy(dst, ptr[:])
                        evac_flip[0] += 1

            load_transpose(q_d, qT, Lq, ICQ, "q")
            load_transpose(k_d, kT, Lkv, ICK, "k")
            load_transpose(v_d, vT, Lkv, ICK, "v")

            # QT[d, q] / KT[d, k]: weights stationary, xT moving
            def project_T(w_sb, x_sb, out_sb, ic_n, L, tag):
                ncols = 512 if L % 512 == 0 else 128
                for sc in range(L // ncols):
                    ps = psp.tile([128, ncols], F32, name=f"ps_{tag}_{sc}", tag="ps")
                    for ic in range(ic_n):
                        nc.tensor.matmul(
                            ps[:],
                            lhsT=w_sb[:, ic, :],
                            rhs=x_sb[:, ic, sc * ncols:(sc + 1) * ncols],
                            start=(ic == 0), stop=(ic == ic_n - 1))
                    nc.vector.tensor_copy(
                        out_sb[:, sc * ncols:(sc + 1) * ncols], ps[:])

            project_T(wq_sb, qT, QT, ICQ, Lq, "q")
            project_T(wk_sb, kT, KT, ICK, Lkv, "k")

            # V natural [k_s, d]: v^T slices stationary, Wv^T moving
            for kt in range(NKT):
                psv = psp.tile([128, D], F32, name=f"psv_{kt}", tag="psv")
                for ic in range(ICK):
                    nc.tensor.matmul(
                        psv[:],
                        lhsT=vT[:, ic, kt * 128:(kt + 1) * 128],
                        rhs=wv_sb[:, ic, :],
                        start=(ic == 0), stop=(ic == ICK - 1))
                nc.vector.tensor_copy(Vn[:, kt, 0:D], psv[:])

        # -------- attention (phase-1 PSUM released first) --------
        pssp = ctx.enter_context(tc.tile_pool(name="psum_s", bufs=2, space="PSUM"))
        psop = ctx.enter_context(tc.tile_pool(name="psum_o", bufs=1, space="PSUM"))
        for qc in range(NQC):
            pso = [psop.tile([128, D + 1], F32, name=f"pso{qs}_{qc}",
                             tag=f"pso{qs}")
                   for qs in range(QSUB)]
            for kt in range(NKT):
                pss = pssp.tile([128, QCW], F32, name=f"pss_{qc}_{kt}",
                                tag="pss")
                # S^T tile [k=128, q=QCW]
                nc.tensor.matmul(
                    pss[:],
                    lhsT=KT[:, kt * 128:(kt + 1) * 128],
                    rhs=QT[:, qc * QCW:(qc + 1) * QCW],
                    start=True, stop=True)
                # P^T = exp(S^T * scale), bf16, no max subtraction
                pt = ptp.tile([128, QCW], BF16, name=f"pt_{qc}_{kt}", tag="pt")
                nc.scalar.activation(
                    pt[:], pss[:], mybir.ActivationFunctionType.Exp,
                    scale=scale)
                for qs in range(QSUB):
                    nc.tensor.matmul(
                        pso[qs][:],
                        lhsT=pt[:, qs * 128:(qs + 1) * 128],
                        rhs=Vn[:, kt, :],
                        start=(kt == 0), stop=(kt == NKT - 1))
            for qs in range(QSUB):
                acc = pso[qs][:]
                r = outp.tile([128, 1], F32, name=f"r_{qc}_{qs}", tag="r")
                nc.vector.reciprocal(r[:], acc[:, D:D + 1])
                o = outp.tile([128, D], F32, name=f"o_{qc}_{qs}", tag="o")
                nc.vector.tensor_scalar_mul(o[:], acc[:, 0:D], r[:])
                row = (qc * QSUB + qs) * 128
                nc.sync.dma_start(out=out_d[:][row:row + 128, :], in_=o[:])


def _get_program():
    key = (LQ, LKV, DQ, DKV)
    if key not in _cache:
        _cache[key] = build_program(*key)
    return _cache[key]


def kernel(q_input, k_input, v_input, Wq, Wk, Wv):
    out_dtype = q_input.dtype
    nc = _get_program()

    wqT = np.ascontiguousarray(Wq.T).astype(ml_dtypes.bfloat16)
    wkT = np.ascontiguousarray(Wk.T).astype(ml_dtypes.bfloat16)
    wvT = np.ascontiguousarray(Wv.T).astype(ml_dtypes.bfloat16)

    in_maps = []
    for c in range(N_CORES):
        in_maps.append({
            "q": np.ascontiguousarray(q_input[c]).astype(np.float32),
            "k": np.ascontiguousarray(k_input[c]).astype(np.float32),
            "v": np.ascontiguousarray(v_input[c]).astype(np.float32),
            "wqT": wqT, "wkT": wkT, "wvT": wvT,
        })

    trace = bool(int(os.environ.get("KERNEL_TRACE", "0")))
    if trace:
        try:
            import antenv.axon_hooks  # noqa: F401  (needed by the trace path)
        except ImportError:
            trace = False
    res = run_bass_kernel_spmd(nc, in_maps, list(range(N_CORES)), trace=trace)
    kernel.last_results = res

    out = np.stack([res.results[c]["out"] for c in range(N_CORES)], axis=0)
    return out.astype(out_dtype)


# revision 17
# speedup vs baseline: 1.3814x; 1.0553x over previous
"""Cross-attention block kernel for 8 TRN2 NeuronCores.

Math (per batch element b, one per core):
    Q = q @ Wq^T            [Lq, 128]
    K = k @ Wk^T            [Lkv, 128]
    V = v @ Wv^T            [Lkv, 128]
    S = Q @ K^T * d^-0.5    [Lq, Lkv]
    O = softmax(S) @ V      [Lq, 128]

Device strategy (per core):
  - HWDGE DMA loads q/k/v fp32 from HBM in 512-row slabs, natural layout.
  - PE transposes (matmul against identity) produce feat-major bf16 tiles;
    the PSUM->SBUF evacuation does the fp32->bf16 cast and is split between
    the vector and scalar engines.
  - Projections: QT/KT computed as [d, seq] (weights stationary); V computed
    in natural [seq, d] layout (v^T tiles stationary, Wv^T moving).
  - S^T tiles [k,q] = KT_slice.T @ QT; softmax runs WITHOUT max subtraction
    (scores are ~N(0,1); exp is safe in fp32) so exp+scale is a single
    scalar-engine activation pass straight out of PSUM.
  - P^T tiles feed PV matmuls as the stationary operand against an
    augmented moving operand [V | 1]: the extra ones column makes the PSUM
    accumulator [q,129] hold both O_unnorm and the softmax denominator.
  - Normalization is a per-partition reciprocal + tensor_scalar multiply.
"""

import os
import numpy as np
import ml_dtypes

from contextlib import ExitStack

import concourse.bass as bass
import concourse.tile as tile
from concourse import bacc, mybir
from concourse.bass_utils import run_bass_kernel_spmd

F32 = mybir.dt.float32
BF16 = mybir.dt.bfloat16

B = 8
LQ = 2048
LKV = 2048
DQ = 512
DKV = 768
D = 128
N_CORES = 8

_cache = {}


def build_program(Lq=LQ, Lkv=LKV, Dq=DQ, Dkv=DKV):
    assert Lq % 128 == 0 and Lkv % 128 == 0 and Dq % 128 == 0 and Dkv % 128 == 0
    nc = bacc.Bacc("TRN2", target_bir_lowering=False)

    q_d = nc.declare_dram_parameter("q", [Lq, Dq], F32, isOutput=False)
    k_d = nc.declare_dram_parameter("k", [Lkv, Dkv], F32, isOutput=False)
    v_d = nc.declare_dram_parameter("v", [Lkv, Dkv], F32, isOutput=False)
    wq_d = nc.declare_dram_parameter("wqT", [Dq, D], BF16, isOutput=False)
    wk_d = nc.declare_dram_parameter("wkT", [Dkv, D], BF16, isOutput=False)
    wv_d = nc.declare_dram_parameter("wvT", [Dkv, D], BF16, isOutput=False)
    out_d = nc.declare_dram_parameter("out", [Lq, D], F32, isOutput=True)

    with tile.TileContext(nc) as tc:
        _body(tc, q_d, k_d, v_d, wq_d, wk_d, wv_d, out_d, Lq, Lkv, Dq, Dkv)
    nc.compile()
    return nc


def _body(tc, q_d, k_d, v_d, wq_d, wk_d, wv_d, out_d, Lq, Lkv, Dq, Dkv):
    nc = tc.nc
    scale = float(D) ** -0.5
    ICQ = Dq // 128   # q feature chunks
    ICK = Dkv // 128  # k/v feature chunks
    NKT = Lkv // 128  # kv seq tiles
    QCW = 512 if Lq % 512 == 0 else 128   # q chunk width for attention
    NQC = Lq // QCW
    QSUB = QCW // 128

    with ExitStack() as ctx:
        # -------- SBUF pools --------
        wpool = ctx.enter_context(tc.tile_pool(name="weights", bufs=1))
        xtp = ctx.enter_context(tc.tile_pool(name="xT", bufs=1))
        projp = ctx.enter_context(tc.tile_pool(name="proj", bufs=1))
        stag = ctx.enter_context(tc.tile_pool(name="stage", bufs=2))
        ptp = ctx.enter_context(tc.tile_pool(name="probs", bufs=3))
        outp = ctx.enter_context(tc.tile_pool(name="outs", bufs=4))

        # weights, already transposed on host: [Din, D] -> sbuf [128, IC, D]
        wq_sb = wpool.tile([128, ICQ, D], BF16, name="wq_sb")
        wk_sb = wpool.tile([128, ICK, D], BF16, name="wk_sb")
        wv_sb = wpool.tile([128, ICK, D], BF16, name="wv_sb")
        nc.scalar.dma_start(out=wq_sb[:], in_=wq_d[:].rearrange("(c p) d -> p c d", p=128))
        nc.scalar.dma_start(out=wk_sb[:], in_=wk_d[:].rearrange("(c p) d -> p c d", p=128))
        nc.scalar.dma_start(out=wv_sb[:], in_=wv_d[:].rearrange("(c p) d -> p c d", p=128))

        # transposed inputs, bf16: xT[p=feat128, chunk, seq]
        qT = xtp.tile([128, ICQ, Lq], BF16, name="qT")
        kT = xtp.tile([128, ICK, Lkv], BF16, name="kT")
        vT = xtp.tile([128, ICK, Lkv], BF16, name="vT")

        # projections
        QT = projp.tile([128, Lq], BF16, name="QT")     # [d, q]
        KT = projp.tile([128, Lkv], BF16, name="KT")    # [d, k]
        Vn = projp.tile([128, NKT, D + 1], BF16, name="Vn")  # natural V + ones

        # identity for PE transposes
        ident = wpool.tile([128, 128], F32, name="ident")
        from concourse.masks import make_identity
        make_identity(nc, ident[:])

        # ones column for the fused denominator trick (data cols overwritten)
        nc.vector.memset(Vn[:], 1.0)

        # -------- load + transpose + projections (phase 1 PSUM scope) ------
        evac_flip = [0]

        with tc.tile_pool(name="psum_tr", bufs=2, space="PSUM") as ptrp, \
             tc.tile_pool(name="psum_proj", bufs=2, space="PSUM") as psp:

            # Load 512-row fp32 slabs, PE-transpose 128x128 tiles into PSUM
            # (4 per bank), evacuate each bank as bf16 (cast in the copy),
            # alternating vector/scalar engines.
            def load_transpose(x_d, xT_sb, n_seq, ic_n, tag, eng=None):
                slab = 1024 if n_seq % 1024 == 0 else (
                    512 if n_seq % 512 == 0 else 128)
                nt = slab // 128
                eng = eng or nc.sync
                for sl in range(n_seq // slab):
                    nat = stag.tile([128, nt, ic_n * 128], F32,
                                    name=f"nat_{tag}_{sl}", tag="nat")
                    eng.dma_start(
                        out=nat[:],
                        in_=x_d[:][sl * slab:(sl + 1) * slab, :]
                        .rearrange("(t p) i -> p t i", p=128))
                    for ic in range(ic_n):
                        ptr = ptrp.tile([128, slab], F32,
                                        name=f"ptr_{tag}_{sl}_{ic}", tag="ptr")
                        for t in range(nt):
                            nc.tensor.transpose(
                                ptr[:, t * 128:(t + 1) * 128],
                                nat[:, t, ic * 128:(ic + 1) * 128],
                                ident[:])
                        dst = xT_sb[:, ic, sl * slab:(sl + 1) * slab]
                        if evac_flip[0] % 2 == 0:
                            nc.vector.tensor_copy(dst, ptr[:])
                        else:
                            nc.scalar.copy(dst, ptr[:])
                        evac_flip[0] += 1

            load_transpose(q_d, qT, Lq, ICQ, "q")
            load_transpose(k_d, kT, Lkv, ICK, "k")
            load_transpose(v_d, vT, Lkv, ICK, "v", eng=nc.scalar)

            # QT[d, q] / KT[d, k]: weights stationary, xT moving
            def project_T(w_sb, x_sb, out_sb, ic_n, L, tag):
                ncols = 512 if L % 512 == 0 else 128
                for sc in range(L // ncols):
                    ps = psp.tile([128, ncols], F32, name=f"ps_{tag}_{sc}", tag="ps")
                    for ic in range(ic_n):
                        nc.tensor.matmul(
                            ps[:],
                            lhsT=w_sb[:, ic, :],
                            rhs=x_sb[:, ic, sc * ncols:(sc + 1) * ncols],
                            start=(ic == 0), stop=(ic == ic_n - 1))
                    nc.vector.tensor_copy(
                        out_sb[:, sc * ncols:(sc + 1) * ncols], ps[:])

            project_T(wq_sb, qT, QT, ICQ, Lq, "q")
            project_T(wk_sb, kT, KT, ICK, Lkv, "k")

            # V natural [k_s, d]: v^T slices stationary, Wv^T moving
            for kt in range(NKT):
                psv = psp.tile([128, D], F32, name=f"psv_{kt}", tag="psv")
                for ic in range(ICK):
                    nc.tensor.matmul(
                        psv[:],
                        lhsT=vT[:, ic, kt * 128:(kt + 1) * 128],
                        rhs=wv_sb[:, ic, :],
                        start=(ic == 0), stop=(ic == ICK - 1))
                nc.vector.tensor_copy(Vn[:, kt, 0:D], psv[:])

        # -------- attention (phase-1 PSUM released first) --------
        pssp = ctx.enter_context(tc.tile_pool(name="psum_s", bufs=2, space="PSUM"))
        psop = ctx.enter_context(tc.tile_pool(name="psum_o", bufs=1, space="PSUM"))
        KTG = 2 if NKT % 2 == 0 else 1   # S^T tiles per exp activation
        for qc in range(NQC):
            pso = [psop.tile([128, D + 1], F32, name=f"pso{qs}_{qc}",
                             tag=f"pso{qs}")
                   for qs in range(QSUB)]
            for g in range(NKT // KTG):
                pss = pssp.tile([128, KTG, QCW], F32, name=f"pss_{qc}_{g}",
                                tag="pss")
                for i in range(KTG):
                    kt = g * KTG + i
                    # S^T tile [k=128, q=QCW]
                    nc.tensor.matmul(
                        pss[:, i, :],
                        lhsT=KT[:, kt * 128:(kt + 1) * 128],
                        rhs=QT[:, qc * QCW:(qc + 1) * QCW],
                        start=True, stop=True)
                # P^T = exp(S^T * scale), bf16, no max subtraction; one
                # activation covers the whole group (halves ACT op overhead)
                pt = ptp.tile([128, KTG, QCW], BF16, name=f"pt_{qc}_{g}",
                              tag="pt")
                nc.scalar.activation(
                    pt[:], pss[:], mybir.ActivationFunctionType.Exp,
                    scale=scale)
                for i in range(KTG):
                    kt = g * KTG + i
                    for qs in range(QSUB):
                        nc.tensor.matmul(
                            pso[qs][:],
                            lhsT=pt[:, i, qs * 128:(qs + 1) * 128],
                            rhs=Vn[:, kt, :],
                            start=(kt == 0), stop=(kt == NKT - 1))
            for qs in range(QSUB):
                acc = pso[qs][:]
                r = outp.tile([128, 1], F32, name=f"r_{qc}_{qs}", tag="r")
                nc.vector.reciprocal(r[:], acc[:, D:D + 1])
                o = outp.tile([128, D], F32, name=f"o_{qc}_{qs}", tag="o")
                nc.vector.tensor_scalar_mul(o[:], acc[:, 0:D], r[:])
                row = (qc * QSUB + qs) * 128
                nc.sync.dma_start(out=out_d[:][row:row + 128, :], in_=o[:])


def _get_program():
    key = (LQ, LKV, DQ, DKV)
    if key not in _cache:
        _cache[key] = build_program(*key)
    return _cache[key]


def kernel(q_input, k_input, v_input, Wq, Wk, Wv):
    out_dtype = q_input.dtype
    nc = _get_program()

    wqT = np.ascontiguousarray(Wq.T).astype(ml_dtypes.bfloat16)
    wkT = np.ascontiguousarray(Wk.T).astype(ml_dtypes.bfloat16)
    wvT = np.ascontiguousarray(Wv.T).astype(ml_dtypes.bfloat16)

    in_maps = []
    for c in range(N_CORES):
        in_maps.append({
            "q": np.ascontiguousarray(q_input[c]).astype(np.float32),
            "k": np.ascontiguousarray(k_input[c]).astype(np.float32),
            "v": np.ascontiguousarray(v_input[c]).astype(np.float32),
            "wqT": wqT, "wkT": wkT, "wvT": wvT,
        })

    trace = bool(int(os.environ.get("KERNEL_TRACE", "0")))
    if trace:
        try:
            import antenv.axon_hooks  # noqa: F401  (needed by the trace path)
        except ImportError:
            trace = False
    res = run_bass_kernel_spmd(nc, in_maps, list(range(N_CORES)), trace=trace)
    kernel.last_results = res

    out = np.stack([res.results[c]["out"] for c in range(N_CORES)], axis=0)
    return out.astype(out_dtype)
